# revision 31
# baseline (speedup 1.0000x reference)
"""Linformer-style multi-head attention on 8 Trainium2 NeuronCores.

Problem (hardcoded): B=4, S=4096, C=1024, H=16, D=64, DK=256, fp32 I/O.

Sharding: core i handles (batch b = i//2, head-group g = i%2 of 8 heads).
Each core computes its 8 heads' attention and the partial output
projection out_part = head_out_g @ Wo[:, g_cols].T; the host sums the two
head-group partials per batch and adds bo.

All matmul operands are bfloat16 (PSUM accumulation fp32): the fp32r
path runs the PE in fp32_mode=HIGH at ~0.55 ns/row sustained, while bf16
streams at the full 0.417 ns/row and halves LDWEIGHTS, DMA and SBUF.
Measured end-to-end rel err vs the fp32 reference is ~7e-3 (tol 2e-2).

Per-core kernel:
  pass 1 (x streamed once in 8 s-chunks of 512):
      K,V = x @ Wk^T, x @ Wv^T        (layout [s, hd])
      Kp[hd,dk]  += K-chunk vs E^T     (PSUM accumulators, full-seq sum)
      VpT[dk,hd] += F^T vs V-chunk     (PSUM accumulators)
      K copies on DVE, V copies on ACT (splits the PSUM-drain load).
  pass 1.5 (x streamed a second time):
      Q^T[hd,s] for all chunks, kept resident in SBUF; these matmuls
      don't depend on Kp/Vp, so the PE stays dense while DVE finalizes
      kp_sb (+E_b) and vpa (+F_b) behind the last accumulation; Q^T
      PSUM drains ride the otherwise-idle ACT engine.
  vpa = per head [ones(64 cols) | VpT(64 cols)] augmented stationary.
  pass 2, software-pipelined over (chunk, head-pair) items (scores+exp
  run 3 items ahead of AV/normalize; a chunk's output projection is
  deferred 2 items so the PE never waits on DVE's normalize):
      scoresT[dk,s] = Kp-slices x Q^T   (row-packed head pairs, K=64)
      expT = exp(scoresT/8) on ACT      (batched [128,1024] per head row)
      per head, ONE matmul group with vpa: rows 0-63 = softmax
          denominator replicated across partitions, rows 64-127 =
          unnormalized head_out^T; then ho = av * recip_approx(denom)
          on DVE (cross-partition-base operands, probed valid on HW)
      out[s,c] = ho-slices x WoT        (accumulate 4 hd blocks; PSUM
          slots shared with the AV pool; copies on ACT)
"""

import threading

import ml_dtypes
import numpy as np

B, S, C = 4, 4096, 1024
H, D, DK = 16, 64, 256
HG = 8               # heads per core
HD = HG * D          # 512
NCORES = 8
SCH = 512            # sequence chunk
NCH = S // SCH       # 8 chunks
NST = SCH // 128     # 4 s-tiles per chunk
NCT = C // 128       # 8 c-tiles
NPT = HD // 128      # 4 hd blocks (head pairs)
NDB = DK // 128      # 2 dk blocks

_lock = threading.Lock()
_compiled = None


def _build():
    import concourse.bacc as bacc
    import concourse.bass as bass
    import concourse.tile as tile
    from concourse import mybir

    F32 = mybir.dt.float32
    BF16 = mybir.dt.bfloat16
    EXP = mybir.ActivationFunctionType.Exp

    nc = bacc.Bacc(None, target_bir_lowering=False)

    xT = nc.dram_tensor("xt", [C, S], BF16, kind="ExternalInput")
    wqT = nc.dram_tensor("wqt", [C, HD], BF16, kind="ExternalInput")
    wkT = nc.dram_tensor("wkt", [C, HD], BF16, kind="ExternalInput")
    wvT = nc.dram_tensor("wvt", [C, HD], BF16, kind="ExternalInput")
    ewT = nc.dram_tensor("ewt", [S, DK], BF16, kind="ExternalInput")
    fwT = nc.dram_tensor("fwt", [S, DK], BF16, kind="ExternalInput")
    eb = nc.dram_tensor("eb", [DK], F32, kind="ExternalInput")
    fb = nc.dram_tensor("fb", [DK], F32, kind="ExternalInput")
    woT = nc.dram_tensor("wot", [HD, C], BF16, kind="ExternalInput")
    ones = nc.dram_tensor("ones", [128, HD], BF16, kind="ExternalInput")
    out = nc.dram_tensor("out", [S, C], BF16, kind="ExternalOutput")

    xT_r = xT[:].rearrange("(ct p) s -> ct p s", p=128)     # [8,128,4096]
    xT_c = xT[:].rearrange("(ct p) s -> p ct s", p=128)     # [128,8,4096]
    wq_c = wqT[:].rearrange("(ct p) n -> p ct n", p=128)    # [128,8,512]
    wk_r = wkT[:].rearrange("(ct p) n -> ct p n", p=128)
    wv_c = wvT[:].rearrange("(ct p) n -> p ct n", p=128)
    ew_c = ewT[:].rearrange("(ch st p) k -> ch p st k", p=128, st=NST)
    fw_c = fwT[:].rearrange("(ch st p) k -> ch p st k", p=128, st=NST)
    wo_c = woT[:].rearrange("(pt p) c -> p pt c", p=128)    # [128,4,1024]

    with tile.TileContext(nc) as tc:
        with (
            tc.tile_pool(name="consts", bufs=1) as consts,
            tc.tile_pool(name="mids", bufs=1) as mids,
        ):
            wq_sb = consts.tile([128, NCT, HD], BF16)
            wk_sb = consts.tile([128, NCT, HD], BF16)
            wv_sb = consts.tile([128, NCT, HD], BF16)
            eb_sb = consts.tile([128, DK], F32)
            fb_sb = consts.tile([128, NDB], F32)

            kp_sb = mids.tile([128, NPT, DK], BF16)     # Kp [hd, dk]
            # Augmented Vp^T: per dk-tile, per head: 64 cols of ones then
            # 64 cols of Vp^T. A single AV matmul then yields rows 0-63 =
            # the softmax denominator replicated across 64 partitions and
            # rows 64-127 = head_out^T (unnormalized).
            vpa_sb = mids.tile([128, NDB, 2 * HD], BF16)
            qt_sb = mids.tile([128, NCH * NPT, SCH], BF16)  # Q^T, all chunks

            # ---------------- pass 1: K/V projections + Kp/VpT ------------
            with (
                tc.tile_pool(name="p1sbuf", bufs=2) as p1s,
                tc.tile_pool(name="p1kv", bufs=1) as p1kv,
                tc.tile_pool(name="p1psum", bufs=2, space="PSUM") as p1ps,
                tc.tile_pool(name="qtpsum", bufs=2, space="PSUM") as qtps,
                tc.tile_pool(name="accps", bufs=1, space="PSUM") as accps,
            ):
                kp_ps = accps.tile([128, NPT, DK], F32)
                vp_ps = accps.tile([128, NDB, HD], F32)
                # chunk-0 x first (per-ct slices so the first matmul can
                # start as soon as ct=0 lands), then weights in use order.
                # Input DMAs split across the two HW DGE queues (Sync and
                # ACT issue into separate queues) to double early HBM pull:
                # sync takes xt0+wk (K path, per-ct so the first matmul
                # starts on slice arrival), scalar takes wv/ew/fw with few
                # issue slots (ACT's stream also runs the V copies), and
                # the not-needed-until-later wq/eb/fb ride gpsimd's
                # software DGE.
                xt0 = p1s.tile([128, NCT, SCH], BF16, name="xt1")
                for ct in range(NCT):
                    nc.sync.dma_start(xt0[:, ct, :], xT_r[ct, :, 0:SCH])
                    nc.sync.dma_start(wk_sb[:, ct, :], wk_r[ct])
                ew0 = p1s.tile([128, NST, DK], BF16, name="ew")
                fw0 = p1s.tile([128, NST, DK], BF16, name="fw")
                wv_r = wvT[:].rearrange("(ct p) n -> ct p n", p=128)
                for ct in range(NCT):
                    nc.sync.dma_start(wv_sb[:, ct, :], wv_r[ct])
                nc.sync.dma_start(ew0[:], ew_c[0])
                nc.sync.dma_start(fw0[:], fw_c[0])
                wq_r = wqT[:].rearrange("(ct p) n -> ct p n", p=128)
                for ct in range(NCT):
                    nc.sync.dma_start(wq_sb[:, ct, :], wq_r[ct])
                eb_bc = bass.AP(tensor=eb[:].tensor, offset=0, ap=[[0, 128], [1, DK]])
                nc.sync.dma_start(eb_sb[:], eb_bc)
                for db in range(NDB):
                    fb_col = fb[db * 128:(db + 1) * 128].rearrange(
                        "(p one) -> p one", one=1
                    )
                    nc.sync.dma_start(fb_sb[:, db:db + 1], fb_col)

                for ch in range(NCH):
                    if ch == 0:
                        xt, ew, fw = xt0, ew0, fw0
                    else:
                        xt = p1s.tile([128, NCT, SCH], BF16, name="xt1")
                        nc.sync.dma_start(
                            xt[:], xT_c[:, :, ch * SCH:(ch + 1) * SCH]
                        )
                        ew = p1s.tile([128, NST, DK], BF16, name="ew")
                        fw = p1s.tile([128, NST, DK], BF16, name="fw")
                        nc.sync.dma_start(ew[:], ew_c[ch])
                        nc.sync.dma_start(fw[:], fw_c[ch])
                    k_sb = p1kv.tile([128, NST, HD], BF16, name="k_sb")
                    v_sb = p1kv.tile([128, NST, HD], BF16, name="v_sb")
                    for st in range(NST):
                        kps = p1ps.tile([128, HD], F32, name="kvps")
                        for ct in range(NCT):
                            nc.tensor.matmul(
                                kps,
                                xt[:, ct, st * 128:(st + 1) * 128],
                                wk_sb[:, ct, :],
                                start=(ct == 0), stop=(ct == NCT - 1),
                            )
                        nc.vector.tensor_copy(k_sb[:, st, :], kps)
                        vps = p1ps.tile([128, HD], F32, name="kvps")
                        for ct in range(NCT):
                            nc.tensor.matmul(
                                vps,
                                xt[:, ct, st * 128:(st + 1) * 128],
                                wv_sb[:, ct, :],
                                start=(ct == 0), stop=(ct == NCT - 1),
                            )
                        nc.scalar.copy(v_sb[:, st, :], vps)
                    first = ch == 0
                    last = ch == NCH - 1
                    for st in range(NST):
                        for pt in range(NPT):
                            # kp_ps slices pt={0,1} share PSUM bank 0 and
                            # pt={2,3} share bank 1 — one accumulation group
                            # per bank: start on the bank's first slice,
                            # stop on its last.
                            nc.tensor.matmul(
                                kp_ps[:, pt, :],
                                k_sb[:, st, pt * 128:(pt + 1) * 128],
                                ew[:, st, :],
                                start=(first and st == 0 and pt % 2 == 0),
                                stop=(last and st == NST - 1 and pt % 2 == 1),
                            )
                        for db in range(NDB):
                            nc.tensor.matmul(
                                vp_ps[:, db, :],
                                fw[:, st, db * 128:(db + 1) * 128],
                                v_sb[:, st, :],
                                start=(first and st == 0),
                                stop=(last and st == NST - 1),
                            )

                # ------------ pass 1.5: Q^T (x streamed a second time) ----
                # Kp/vpa finalize on DVE runs under the Q^T matmuls; Q^T
                # PSUM drains go to ACT so the two never queue behind each
                # other.
                for pt in range(NPT):
                    nc.vector.tensor_add(kp_sb[:, pt, :], kp_ps[:, pt, :], eb_sb)
                ones_r = ones[:].rearrange("p (h d) -> p h d", d=64)
                for db in range(NDB):
                    vpa_v = vpa_sb[:, db, :].rearrange(
                        "p (h two d) -> p h two d", two=2, d=64
                    )
                    nc.gpsimd.dma_start(vpa_v[:, :, 0, :], ones_r)
                    nc.vector.tensor_scalar_add(
                        vpa_v[:, :, 1, :],
                        vp_ps[:, db, :].rearrange("p (h d) -> p h d", d=64),
                        fb_sb[:, db:db + 1],
                    )
                for ch in range(NCH):
                    xt = p1s.tile([128, NCT, SCH], BF16, name="xt1")
                    nc.sync.dma_start(xt[:], xT_c[:, :, ch * SCH:(ch + 1) * SCH])
                    for pt in range(NPT):
                        qps = qtps.tile([128, SCH], F32, name="qps")
                        for ct in range(NCT):
                            nc.tensor.matmul(
                                qps,
                                wq_sb[:, ct, pt * 128:(pt + 1) * 128],
                                xt[:, ct, :],
                                start=(ct == 0), stop=(ct == NCT - 1),
                            )
                        nc.scalar.copy(qt_sb[:, ch * NPT + pt, :], qps)

            # ---------------- pass 2: attention + output projection -------
            with (
                tc.tile_pool(name="p2wo", bufs=1) as p2wo,
                tc.tile_pool(name="p2ex", bufs=3) as p2ex,
                tc.tile_pool(name="p2ho", bufs=2) as p2ho,
                tc.tile_pool(name="p2rc", bufs=1) as p2rc,
                tc.tile_pool(name="p2out", bufs=2) as p2out,
                tc.tile_pool(name="scps", bufs=2, space="PSUM") as scps,
                # av tiles (attention) and output-projection accumulators
                # share one 4-slot pool via a common tag.
                tc.tile_pool(name="avout", bufs=4, space="PSUM") as avout,
            ):
                wo_sb = p2wo.tile([128, NPT, C], BF16)
                nc.sync.dma_start(wo_sb[:], wo_c)
                ho_tiles = {}

                # Work units: full 512-wide chunks except the last chunk,
                # processed as two 256-wide halves so the final out-DMA
                # drain (write bandwidth ~130 GB/s) covers 0.5 MB, not 1 MB.
                keys = [(ch, 0, SCH) for ch in range(NCH - 1)]
                keys += [(NCH - 1, 0, SCH // 2), (NCH - 1, SCH // 2, SCH // 2)]

                def stage_a(key, pt):
                    ch, s0, sw = key
                    qt_c = qt_sb[:, ch * NPT + pt, s0:s0 + sw]
                    ex = p2ex.tile([128, 2, NDB, SCH], BF16, name="ex")
                    for hrow in range(2):
                        lo, hi = hrow * 64, (hrow + 1) * 64
                        scp = scps.tile([128, NDB, SCH], F32, name="scp")
                        for j in range(NDB):
                            nc.tensor.matmul(
                                scp[:, j, 0:sw],
                                kp_sb[lo:hi, pt, j * 128:(j + 1) * 128],
                                qt_c[lo:hi, :],
                                start=True, stop=True,
                            )
                        nc.scalar.activation(
                            ex[:, hrow, :, 0:sw], scp[:, :, 0:sw],
                            EXP, scale=0.125,
                        )
                    return ex

                def stage_b(key, pt, ex):
                    # per-head: one matmul group with the augmented
                    # [ones | VpT] stationary operand gives the replicated
                    # denominator (rows 0-63) and unnormalized AV (rows
                    # 64-127) in one PSUM tile; then reciprocal + multiply.
                    ch, s0, sw = key
                    if pt == 0:
                        ho_tiles[key] = p2ho.tile(
                            [128, NPT, SCH], BF16, name="ho_sb"
                        )
                    ho_sb = ho_tiles[key]
                    for hrow in range(2):
                        a0 = pt * 256 + hrow * 128
                        av = avout.tile([128, SCH], F32, name="avout")
                        for kt in range(NDB):
                            nc.tensor.matmul(
                                av[:, 0:sw],
                                vpa_sb[:, kt, a0:a0 + 128],
                                ex[:, hrow, kt, 0:sw],
                                start=(kt == 0), stop=(kt == NDB - 1),
                            )
                        # rows 0-63 = replicated denominator, rows 64-127 =
                        # unnormalized AV (vpa is [ones | VpT] per head), so
                        # the custom-DVE reciprocal stays fully at base 0 —
                        # custom ops misread partitions at nonzero bases.
                        rc = p2rc.tile([64, SCH], F32, name="rc")
                        nc.vector.reciprocal_approx_fast(
                            rc[:, 0:sw], av[0:64, 0:sw]
                        )
                        lo = hrow * 64
                        nc.vector.tensor_mul(
                            ho_sb[lo:lo + 64, pt, 0:sw],
                            av[64:128, 0:sw], rc[:, 0:sw],
                        )

                def outproj(key):
                    ch, s0, sw = key
                    ho_sb = ho_tiles.pop(key)
                    last = ch == NCH - 1
                    for st in range(sw // 128):
                        osb = p2out.tile([128, C], BF16, name="osb")
                        for cc in range(2):
                            ops = avout.tile([128, SCH], F32, name="avout")
                            for pt in range(NPT):
                                nc.tensor.matmul(
                                    ops[:, 0:512],
                                    ho_sb[:, pt, st * 128:(st + 1) * 128],
                                    wo_sb[:, pt, cc * 512:(cc + 1) * 512],
                                    start=(pt == 0), stop=(pt == NPT - 1),
                                )
                            # PSUM drains split ACT/DVE so the final
                            # chunk's copies don't serialize on one engine.
                            cols = slice(cc * 512, (cc + 1) * 512)
                            if cc == 0:
                                nc.scalar.copy(osb[:, cols], ops[:, 0:512])
                            else:
                                nc.vector.tensor_copy(
                                    osb[:, cols], ops[:, 0:512]
                                )
                        row = ch * SCH + s0 + st * 128
                        # last chunk: drain the tail on both DGE queues
                        eng = nc.scalar if (last and st % 2) else nc.sync
                        eng.dma_start(out[row:row + 128, :], osb)

                items = [(key, pt) for key in keys for pt in range(NPT)]
                last_idx = {key: i * NPT + NPT - 1 for i, key in enumerate(keys)}
                # outproj(key) lands two items after key's last stage_b: by
                # then its last DVE normalize has drained behind the
                # interleaved scores/AV matmuls.
                oproj_at = {last_idx[key] + 2: key for key in keys}
                DEPTH = 3
                ex_tiles = {}
                for i in range(DEPTH):
                    ex_tiles[items[i]] = stage_a(*items[i])
                for i, (key, pt) in enumerate(items):
                    if i + DEPTH < len(items):
                        ex_tiles[items[i + DEPTH]] = stage_a(*items[i + DEPTH])
                    stage_b(key, pt, ex_tiles.pop((key, pt)))
                    if i in oproj_at:
                        outproj(oproj_at[i])
                for i in sorted(oproj_at):
                    if i >= len(items):
                        outproj(oproj_at[i])

    nc.compile()
    return nc


def get_compiled():
    global _compiled
    with _lock:
        if _compiled is None:
            _compiled = _build()
    return _compiled


def make_in_maps(x, Wq, Wk, Wv, E_w, E_b, F_w, F_b, Wo, bo):
    """Host-side sharding: core i -> (batch i//2, head-group i%2)."""
    f = np.float32
    bf = ml_dtypes.bfloat16
    x = np.asarray(x, f)
    ewT = np.ascontiguousarray(np.asarray(E_w, f).T.astype(bf))    # [S, DK]
    fwT = np.ascontiguousarray(np.asarray(F_w, f).T.astype(bf))
    in_maps = []
    for core in range(NCORES):
        b, g = divmod(core, 2)
        hs = slice(g * HG, (g + 1) * HG)
        wq = np.asarray(Wq, f)[hs].reshape(HD, C)
        wk = np.asarray(Wk, f)[hs].reshape(HD, C)
        wv = np.asarray(Wv, f)[hs].reshape(HD, C)
        wo = np.asarray(Wo, f)[:, g * HD:(g + 1) * HD]      # [C, 512]
        in_maps.append({
            "xt": np.ascontiguousarray(x[b].T.astype(bf)),  # [C, S]
            "wqt": np.ascontiguousarray(wq.T.astype(bf)),   # [C, HD]
            "wkt": np.ascontiguousarray(wk.T.astype(bf)),
            "wvt": np.ascontiguousarray(wv.T.astype(bf)),
            "ewt": ewT,
            "fwt": fwT,
            "eb": np.asarray(E_b, f),
            "fb": np.asarray(F_b, f),
            "wot": np.ascontiguousarray(wo.T.astype(bf)),   # [HD, C]
            "ones": np.ones((128, HD), bf),
        })
    return in_maps


def assemble(results, bo):
    out = np.empty((B, S, C), np.float32)
    for b in range(B):
        out[b] = results[2 * b]["out"].astype(np.float32) + \
            results[2 * b + 1]["out"].astype(np.float32)
    out += np.asarray(bo, np.float32)[None, None, :]
    return out


def kernel(x, Wq, Wk, Wv, E_w, E_b, F_w, F_b, Wo, bo):
    from concourse.bass_utils import run_bass_kernel_spmd

    nc = get_compiled()
    in_maps = make_in_maps(x, Wq, Wk, Wv, E_w, E_b, F_w, F_b, Wo, bo)
    res = run_bass_kernel_spmd(nc, in_maps, core_ids=list(range(NCORES)))
    return assemble(res.results, bo)
